# revision 4
# baseline (speedup 1.0000x reference)
"""Attention kernel for Trainium2, 8 NeuronCores.

Reference computation (per batch b, head h):
    sim  = q @ k^T * D**-0.5         [S, S]
    attn = softmax(sim, axis=-1)
    out  = attn @ v                  [S, D]

Sharding: B*H = 32 (batch, head) pairs are split 4-per-core across 8 cores;
each core computes full attention for its 4 heads independently (no
collectives).

Per-head on-device algorithm (all matmuls bf16 inputs, f32 PSUM accum):
  - q,k,v loaded with SWDGE cast-DMA f32->bf16 into [128, 16*64] natural
    layout (partition = s mod 128, free = (s_chunk, d)).
  - qT, kT [64, S] built via TensorE transposes (contraction dim d must be
    on partitions for the QK^T matmul).
  - For each j-chunk (128 rows of k): scoresT[j, i] = sum_d kT[d,j] qT[d,i]
    computed into PSUM [128, 1024] halves; ACT applies exp(scale*x)
    (unsafe softmax, no max subtraction -- scores are ~N(0,1) after scale,
    bounded by ~6, exp stays in f32/bf16 range) writing bf16 P^T to SBUF.
  - PV: P^T[j, i-chunk] is the *stationary* operand, moving operand is
    v2 = [v | 1] (ones column appended), accumulating over j-chunks into
    natural-layout PSUM out[i, 65]: cols 0..63 = unnormalized out,
    col 64 = softmax denominator l[i].
  - DVE: reciprocal of l + per-partition tensor_scalar multiply -> f32 out.
"""

import os
import sys
from contextlib import ExitStack

sys.path.insert(0, "/opt/trn_rl_repo")

import numpy as np

import concourse.bass as bass
import concourse.mybir as mybir
import concourse.tile as tile
from concourse import bacc
from concourse.masks import make_identity

B, H, S, D = 2, 16, 2048, 64
N_CORES = 8
HPC = (B * H) // N_CORES  # heads per core = 4
NCH = S // 128  # 16 chunks of 128 along S
BF16 = mybir.dt.bfloat16
F32 = mybir.dt.float32
SCALE = float(D) ** -0.5
W = D + 1  # 65: v columns + ones column

_CACHED_NC = None
_LAST_RESULTS = None  # BassKernelResults of the most recent run (for test.py)


def build_attention_bass():
    nc = bacc.Bacc("TRN2", target_bir_lowering=False, debug=False)
    q = nc.dram_tensor("q", [HPC, S, D], F32, kind="ExternalInput").ap()
    k = nc.dram_tensor("k", [HPC, S, D], F32, kind="ExternalInput").ap()
    v = nc.dram_tensor("v", [HPC, S, D], F32, kind="ExternalInput").ap()
    out = nc.dram_tensor("out", [HPC, S, D], F32, kind="ExternalOutput").ap()

    with tile.TileContext(nc) as tc, ExitStack() as ctx:
        const = ctx.enter_context(tc.tile_pool(name="const", bufs=1))
        loads = ctx.enter_context(tc.tile_pool(name="loads", bufs=2))
        v2p = ctx.enter_context(tc.tile_pool(name="v2p", bufs=2))
        qkp = ctx.enter_context(tc.tile_pool(name="qkp", bufs=2))
        ptp = ctx.enter_context(tc.tile_pool(name="ptp", bufs=4))
        outp = ctx.enter_context(tc.tile_pool(name="outp", bufs=2))
        rcpp = ctx.enter_context(tc.tile_pool(name="rcpp", bufs=2))
        # PSUM pools: scores 2 banks x2 bufs + accum 3 banks + transpose 1 = 8
        scp = ctx.enter_context(tc.tile_pool(name="scp", bufs=2, space="PSUM"))
        accp = ctx.enter_context(tc.tile_pool(name="accp", bufs=1, space="PSUM"))
        tps = ctx.enter_context(tc.tile_pool(name="tps", bufs=1, space="PSUM"))

        ident = const.tile([128, 128], BF16)
        make_identity(nc, ident)

        for h in range(HPC):
            # ---- loads (cast f32 -> bf16 during DMA; SWDGE) ----
            q_nat = loads.tile([128, NCH * D], BF16, tag="qnat")
            nc.gpsimd.dma_start(
                out=q_nat.rearrange("p (c d) -> p c d", d=D),
                in_=q[h].rearrange("(c p) d -> p c d", p=128),
            )
            k_nat = loads.tile([128, NCH * D], BF16, tag="knat")
            nc.gpsimd.dma_start(
                out=k_nat.rearrange("p (c d) -> p c d", d=D),
                in_=k[h].rearrange("(c p) d -> p c d", p=128),
            )
            v_nat = loads.tile([128, NCH * D], BF16, tag="vnat")
            nc.gpsimd.dma_start(
                out=v_nat.rearrange("p (c d) -> p c d", d=D),
                in_=v[h].rearrange("(c p) d -> p c d", p=128),
            )

            # ---- v2 = [v | 1] per j-chunk ----
            v2 = v2p.tile([128, NCH * W], BF16, tag="v2")
            v2_3d = v2.rearrange("p (c w) -> p c w", w=W)
            nc.vector.memset(v2_3d[:, :, D : D + 1], 1.0)
            nc.vector.tensor_copy(
                v2_3d[:, :, 0:D], v_nat.rearrange("p (c d) -> p c d", d=D)
            )

            # ---- qT, kT via TensorE transposes ----
            qT = qkp.tile([64, S], BF16, tag="qT")
            kT = qkp.tile([64, S], BF16, tag="kT")
            for src_nat, dstT in ((q_nat, qT), (k_nat, kT)):
                for b in range(4):  # 4 batches x 4 chunks
                    tp = tps.tile([64, 512], BF16, tag="tpsum")
                    for j in range(4):
                        c = b * 4 + j
                        nc.tensor.transpose(
                            out=tp[:, j * 128 : (j + 1) * 128],
                            in_=src_nat[:, c * D : (c + 1) * D],
                            identity=ident,
                        )
                    nc.vector.tensor_copy(dstT[:, b * 512 : (b + 1) * 512], tp)

            # ---- main loop over j-chunks ----
            accum = accp.tile([128, 3, 512], F32, tag="accum")
            for jc in range(NCH):
                kslc = kT[:, jc * 128 : (jc + 1) * 128]
                for half in range(2):
                    sc_ps = scp.tile([128, 1024], F32, tag="scores")
                    for n in range(2):
                        nc.tensor.matmul(
                            sc_ps[:, n * 512 : (n + 1) * 512],
                            lhsT=kslc,
                            rhs=qT[
                                :,
                                half * 1024 + n * 512 : half * 1024 + (n + 1) * 512,
                            ],
                            start=True,
                            stop=True,
                        )
                    pt = ptp.tile([128, 1024], BF16, tag="pt")
                    nc.scalar.activation(
                        pt, sc_ps, mybir.ActivationFunctionType.Exp, scale=SCALE
                    )
                    for i8 in range(8):
                        ic = half * 8 + i8
                        bank, slot = divmod(ic, 7)
                        # start/stop once per *bank*: start=True clears the
                        # has_written bits for the whole 2KB bank, so only the
                        # first chunk in each bank may issue it; later chunks'
                        # first write lands on cleared bits -> overwrite+set.
                        nc.tensor.matmul(
                            accum[:, bank, slot * W : (slot + 1) * W],
                            lhsT=pt[:, i8 * 128 : (i8 + 1) * 128],
                            rhs=v2_3d[:, jc, :],
                            start=(jc == 0 and slot == 0),
                            stop=(jc == NCH - 1 and (slot == 6 or ic == NCH - 1)),
                        )

            # ---- normalize + store ----
            out_sb = outp.tile([128, NCH * D], F32, tag="outsb")
            rcp = rcpp.tile([128, NCH], F32, tag="rcp")
            for ic in range(NCH):
                bank, slot = divmod(ic, 7)
                nc.vector.reciprocal(
                    rcp[:, ic : ic + 1],
                    accum[:, bank, slot * W + D : slot * W + W],
                )
                nc.vector.tensor_scalar_mul(
                    out_sb[:, ic * D : (ic + 1) * D],
                    accum[:, bank, slot * W : slot * W + D],
                    rcp[:, ic : ic + 1],
                )
            nc.sync.dma_start(
                out=out[h].rearrange("(c p) d -> p c d", p=128),
                in_=out_sb.rearrange("p (c d) -> p c d", d=D),
            )

    nc.compile()
    return nc


def _get_nc():
    global _CACHED_NC
    if _CACHED_NC is None:
        _CACHED_NC = build_attention_bass()
    return _CACHED_NC


def kernel(q: np.ndarray, k: np.ndarray, v: np.ndarray) -> np.ndarray:
    """Full inputs [B, H, S, D] f32 -> full output [B, H, S, D] f32."""
    global _LAST_RESULTS
    from concourse.bass_utils import run_bass_kernel_spmd

    nc = _get_nc()
    qf = np.ascontiguousarray(np.asarray(q, dtype=np.float32)).reshape(B * H, S, D)
    kf = np.ascontiguousarray(np.asarray(k, dtype=np.float32)).reshape(B * H, S, D)
    vf = np.ascontiguousarray(np.asarray(v, dtype=np.float32)).reshape(B * H, S, D)

    in_maps = []
    for c in range(N_CORES):
        sl = slice(c * HPC, (c + 1) * HPC)
        in_maps.append(
            {
                "q": np.ascontiguousarray(qf[sl]),
                "k": np.ascontiguousarray(kf[sl]),
                "v": np.ascontiguousarray(vf[sl]),
            }
        )

    res = run_bass_kernel_spmd(nc, in_maps, core_ids=list(range(N_CORES)))
    _LAST_RESULTS = res
    outs = [res.results[c]["out"] for c in range(N_CORES)]
    full = np.concatenate(outs, axis=0).reshape(B, H, S, D)
    return full.astype(np.float32)


# revision 7
# speedup vs baseline: 1.1990x; 1.1990x over previous
"""Attention kernel for Trainium2, 8 NeuronCores.

Reference computation (per batch b, head h):
    sim  = q @ k^T * D**-0.5         [S, S]
    attn = softmax(sim, axis=-1)
    out  = attn @ v                  [S, D]

Sharding: B*H = 32 (batch, head) pairs are split 4-per-core across 8 cores;
each core computes full attention for its 4 heads independently (no
collectives).

Per-core algorithm (bf16 matmul inputs, f32 PSUM accumulation):
  Prologue (all 4 heads):
    - q,k,v loaded with SWDGE cast-DMA f32->bf16, natural [128, 16*64]
      layout (partition = s mod 128).
    - qTd,kTd [128, S]: d-major transposed copies, duplicated onto both
      partition halves (rows 0-63 and 64-127 hold the same [64, S] data) so
      QK^T can run 2 j-chunks concurrently in the PE array's two row-group
      halves (K=64 row packing).
    - v2 = [v | 1] (ones column appended per j-chunk).
  Main loop, per head, per i-half (1024 columns), per j-chunk-pair:
    - scoresT psum [128, 2, 512]: slot s = j-chunk jc0+s; row-packed matmuls
      lhsT=kTd[64s:64s+64, jc], rhs=qTd[64s:64s+64, i-cols] (tile_position
      auto-derived from base partition).
    - ACT: exp(scale*x) over the [2, 512] free dims -> bf16 P^T in SBUF
      (unsafe softmax: scores ~N(0,1) after scale, |s| < ~6).
    - PV: stationary v2[jc] [128 j, 65], moving P^T [128 j, 512 i] ->
      accumT psum [65, 1024]: rows 0-63 = out^T unnormalized, row 64 =
      softmax denominator l[i] (free via the ones column).
  Per i-half epilogue: DVE copy accumT->SBUF f32, PE transpose-back
  ([65,128] -> [128,65] via f32 identity), DVE reciprocal of col 64 +
  per-partition tensor_scalar multiply -> natural f32 out, DMA out.
"""

import os
import sys
from contextlib import ExitStack

sys.path.insert(0, "/opt/trn_rl_repo")

import numpy as np

import concourse.bass as bass
import concourse.mybir as mybir
import concourse.tile as tile
from concourse import bacc
from concourse.masks import make_identity

B, H, S, D = 2, 16, 2048, 64
N_CORES = 8
HPC = (B * H) // N_CORES  # heads per core = 4
NCH = S // 128  # 16 chunks of 128 along S
BF16 = mybir.dt.bfloat16
F32 = mybir.dt.float32
SCALE = float(D) ** -0.5
W = D + 1  # 65: v columns + ones column

_CACHED_NC = None
_LAST_RESULTS = None  # BassKernelResults of the most recent run (for test.py)

# Build-time feature flags (for HW bisection)
ROW_PACK = True
TBACK = True


def build_attention_bass():
    nc = bacc.Bacc("TRN2", target_bir_lowering=False, debug=False)
    q = nc.dram_tensor("q", [HPC, S, D], F32, kind="ExternalInput").ap()
    k = nc.dram_tensor("k", [HPC, S, D], F32, kind="ExternalInput").ap()
    v = nc.dram_tensor("v", [HPC, S, D], F32, kind="ExternalInput").ap()
    out = nc.dram_tensor("out", [HPC, S, D], F32, kind="ExternalOutput").ap()

    with tile.TileContext(nc) as tc, ExitStack() as ctx:
        const = ctx.enter_context(tc.tile_pool(name="const", bufs=1))
        loads = ctx.enter_context(tc.tile_pool(name="loads", bufs=HPC))
        v2p = ctx.enter_context(tc.tile_pool(name="v2p", bufs=HPC))
        qkp = ctx.enter_context(tc.tile_pool(name="qkp", bufs=HPC))
        ptp = ctx.enter_context(tc.tile_pool(name="ptp", bufs=4))
        outtp = ctx.enter_context(tc.tile_pool(name="outtp", bufs=2))
        outp = ctx.enter_context(tc.tile_pool(name="outp", bufs=2))
        rcpp = ctx.enter_context(tc.tile_pool(name="rcpp", bufs=2))
        # PSUM: scores 2 banks x2 + accumT 2 banks x1 + scratch 1 bank x2 = 8
        scp = ctx.enter_context(tc.tile_pool(name="scp", bufs=2, space="PSUM"))
        accp = ctx.enter_context(tc.tile_pool(name="accp", bufs=1, space="PSUM"))
        tps = ctx.enter_context(tc.tile_pool(name="tps", bufs=2, space="PSUM"))

        ident = const.tile([128, 128], BF16)
        make_identity(nc, ident)
        identf = const.tile([128, 128], F32)
        make_identity(nc, identf)

        # ---------------- prologue: loads + transposes, all heads ----------
        v2s, qTds, kTds = [], [], []
        for h in range(HPC):
            q_nat = loads.tile([128, NCH * D], BF16, tag="qnat")
            nc.gpsimd.dma_start(
                out=q_nat.rearrange("p (c d) -> p c d", d=D),
                in_=q[h].rearrange("(c p) d -> p c d", p=128),
            )
            k_nat = loads.tile([128, NCH * D], BF16, tag="knat")
            nc.gpsimd.dma_start(
                out=k_nat.rearrange("p (c d) -> p c d", d=D),
                in_=k[h].rearrange("(c p) d -> p c d", p=128),
            )
            v_nat = loads.tile([128, NCH * D], BF16, tag="vnat")
            nc.gpsimd.dma_start(
                out=v_nat.rearrange("p (c d) -> p c d", d=D),
                in_=v[h].rearrange("(c p) d -> p c d", p=128),
            )

            v2 = v2p.tile([128, NCH * W], BF16, tag="v2")
            v2_3d = v2.rearrange("p (c w) -> p c w", w=W)
            nc.vector.memset(v2_3d[:, :, D : D + 1], 1.0)
            nc.vector.tensor_copy(
                v2_3d[:, :, 0:D], v_nat.rearrange("p (c d) -> p c d", d=D)
            )
            v2s.append(v2_3d)

            # qTd/kTd: [128, S] with the [64, S] transposed tensor duplicated
            # onto both partition halves (for K=64 row packing).
            qTd = qkp.tile([128, S], BF16, tag="qTd")
            kTd = qkp.tile([128, S], BF16, tag="kTd")
            for src_nat, dstT in ((q_nat, qTd), (k_nat, kTd)):
                for b in range(4):  # 4 batches x 4 chunks of 128 cols
                    tp = tps.tile([64, 512], BF16, tag="scratch")
                    for j in range(4):
                        c = b * 4 + j
                        nc.tensor.transpose(
                            out=tp[:, j * 128 : (j + 1) * 128],
                            in_=src_nat[:, c * D : (c + 1) * D],
                            identity=ident,
                        )
                    nc.vector.tensor_copy(dstT[0:64, b * 512 : (b + 1) * 512], tp)
                if ROW_PACK:
                    # duplicate to partitions 64-127: DVE cannot move data
                    # across partitions (lanes are partition-locked); DMA can.
                    nc.sync.dma_start(out=dstT[64:128, :], in_=dstT[0:64, :])
            qTds.append(qTd)
            kTds.append(kTd)

        # ---------------- main loops ---------------------------------------
        for h in range(HPC):
            v2_3d, qTd, kTd = v2s[h], qTds[h], kTds[h]
            out_sb = outp.tile([128, NCH * D], F32, tag="outsb")
            for ihalf in range(2):
                icol0 = ihalf * 1024
                accumT = accp.tile([65, 1024], F32, tag="accumT")
                for jcp in range(NCH // 2):
                    jc0 = 2 * jcp
                    pts = []
                    for n in range(2):
                        sc = scp.tile([128, 2, 512], F32, tag="scores")
                        for s in range(2):
                            jc = jc0 + s
                            ro = 64 * s if ROW_PACK else 0
                            nc.tensor.matmul(
                                sc[:, s, :],
                                lhsT=kTd[ro : ro + 64, jc * 128 : (jc + 1) * 128],
                                rhs=qTd[
                                    ro : ro + 64,
                                    icol0 + n * 512 : icol0 + (n + 1) * 512,
                                ],
                                start=True,
                                stop=True,
                            )
                        pt = ptp.tile([128, 2, 512], BF16, tag="pt")
                        nc.scalar.activation(
                            pt, sc, mybir.ActivationFunctionType.Exp, scale=SCALE
                        )
                        pts.append(pt)
                    for s in range(2):
                        jc = jc0 + s
                        for n in range(2):
                            nc.tensor.matmul(
                                accumT[:, n * 512 : (n + 1) * 512],
                                lhsT=v2_3d[:, jc, :],
                                rhs=pts[n][:, s, :],
                                start=(jcp == 0 and s == 0),
                                stop=(jcp == NCH // 2 - 1 and s == 1),
                            )

                # epilogue for this i-half: drain, transpose back, normalize
                outT_sb = outtp.tile([65, 1024], F32, tag="outTsb")
                nc.vector.tensor_copy(outT_sb, accumT)
                if not TBACK:
                    nc.vector.tensor_copy(
                        out_sb[0:65, icol0 : icol0 + 1024], outT_sb
                    )
                    continue
                rcp = rcpp.tile([128, 8], F32, tag="rcp")
                for tb_b in range(2):  # 2 batches x 4 chunks of 128 i
                    tb = tps.tile([128, 4, W], F32, tag="scratch")
                    for j in range(4):
                        c = tb_b * 4 + j
                        nc.tensor.transpose(
                            out=tb[:, j, :],
                            in_=outT_sb[:, c * 128 : (c + 1) * 128],
                            identity=identf[0:65, 0:65],
                        )
                    for j in range(4):
                        ic = ihalf * 8 + tb_b * 4 + j
                        ri = tb_b * 4 + j
                        nc.vector.reciprocal(rcp[:, ri : ri + 1], tb[:, j, D : D + 1])
                        nc.vector.tensor_scalar_mul(
                            out_sb[:, ic * D : (ic + 1) * D],
                            tb[:, j, 0:D],
                            rcp[:, ri : ri + 1],
                        )
            nc.sync.dma_start(
                out=out[h].rearrange("(c p) d -> p c d", p=128),
                in_=out_sb.rearrange("p (c d) -> p c d", d=D),
            )

    nc.compile()
    return nc


def _get_nc():
    global _CACHED_NC
    if _CACHED_NC is None:
        _CACHED_NC = build_attention_bass()
    return _CACHED_NC


def kernel(q: np.ndarray, k: np.ndarray, v: np.ndarray) -> np.ndarray:
    """Full inputs [B, H, S, D] f32 -> full output [B, H, S, D] f32."""
    global _LAST_RESULTS
    from concourse.bass_utils import run_bass_kernel_spmd

    nc = _get_nc()
    qf = np.ascontiguousarray(np.asarray(q, dtype=np.float32)).reshape(B * H, S, D)
    kf = np.ascontiguousarray(np.asarray(k, dtype=np.float32)).reshape(B * H, S, D)
    vf = np.ascontiguousarray(np.asarray(v, dtype=np.float32)).reshape(B * H, S, D)

    in_maps = []
    for c in range(N_CORES):
        sl = slice(c * HPC, (c + 1) * HPC)
        in_maps.append(
            {
                "q": np.ascontiguousarray(qf[sl]),
                "k": np.ascontiguousarray(kf[sl]),
                "v": np.ascontiguousarray(vf[sl]),
            }
        )

    res = run_bass_kernel_spmd(nc, in_maps, core_ids=list(range(N_CORES)))
    _LAST_RESULTS = res
    outs = [res.results[c]["out"] for c in range(N_CORES)]
    full = np.concatenate(outs, axis=0).reshape(B, H, S, D)
    return full.astype(np.float32)
